# revision 77
# baseline (speedup 1.0000x reference)
"""RBF causal attention (unnormalized, no softmax denominator) on 8 Trainium2 NeuronCores.

Problem: B=2 H=16 N=2048 D=128 fp32.
  P[m,n] = exp(-s*||q_m - k_n||^2) for m >= n else 0;  O = P @ V
         = exp(2s*(q.k) - s*|q|^2 - s*|k|^2) masked causally.

Sharding: (b*h) = 32 independent slices -> 4 per core across 8 cores, no comms.

v4: bf16 matmuls, multi-engine rebalance, deep cross-slice/cross-pass
software pipelining. Per (b,h) slice:
  - load Q,K (f32r bits) and V n-major [128, 16, 128]
  - PE-transpose Q,K (f32r, exact) via PSUM; DVE drains convert to bf16
    -> QT,KT [d=128, n] bf16
  - k_sq: DVE half-block squares + reduces (low-latency path for the exp
    bias); q_sq: Pool squares + DVE reduce (slack path for the final scale);
    ksqb=-s*k_sq (Pool); eq=exp(-s*q_sq) (ACT, inside own pass-0 stream);
    V -> bf16 (Pool copy)
  - two m-passes; per key block bn:
      ST[n, m] = KT_bn^T @ QT chunk  (bf16 matmul, f32 PSUM)
      diag: ST += {-30000 where m<n} via identity matmul (mask in exponent)
      PT[n, m] = exp(2s*ST - s*k_sq[n])  (ACT, bf16 out, per-partition bias)
    per query block bm (PT stationary -> output lands naturally as [m, d]):
      O[m, d] += PT_bn^T(cols bm) @ V_bn  (bf16, f32 PSUM accum over bn)
  - O drain: DVE multiplies by eq[m] (per-partition) PSUM->SBUF f32, DMA out.
Pipelining: next slice's transposes + k_sq chain are woven into this slice's
pass-1 QK stream (PE/DVE filler during the ACT-bound stretch); pass 1's first
QK/exp is emitted before pass 0's last PVs so ACT never idles at the pass
boundary.
"""

import os
import sys

import numpy as np

_TRN_REPO = "/opt/trn_rl_repo"
if os.path.isdir(_TRN_REPO) and _TRN_REPO not in sys.path:
    sys.path.insert(0, _TRN_REPO)

import concourse.bass as bass  # noqa: E402
import concourse.mybir as mybir  # noqa: E402
import concourse.tile as tile  # noqa: E402
from concourse import bacc  # noqa: E402
from concourse.bass_utils import run_bass_kernel_spmd  # noqa: E402
from concourse.masks import make_identity, make_upper_triangular  # noqa: E402

B, H, N, D = 2, 16, 2048, 128
SM_SCALE = 0.08838834764831845  # 1/sqrt(D)
NCORES = 8
SLICES = (B * H) // NCORES  # per core
NT = N // 128  # 16 row-blocks per slice

F32 = mybir.dt.float32
F32R = mybir.dt.float32r
BF16 = mybir.dt.bfloat16
MASKVAL = -30000.0  # 2s*(-30000) ~ -5300 in the exponent -> exp == 0

_nc_cache = None


def _build_nc():
    nc = bacc.Bacc("TRN2", target_bir_lowering=False, debug=False, num_devices=NCORES)

    q_dram = nc.dram_tensor("q", [SLICES, N, D], F32R, kind="ExternalInput").ap()
    k_dram = nc.dram_tensor("k", [SLICES, N, D], F32R, kind="ExternalInput").ap()
    v_dram = nc.dram_tensor("v", [SLICES, N, D], F32, kind="ExternalInput").ap()
    o_dram = nc.dram_tensor("o", [SLICES, N, D], F32, kind="ExternalOutput").ap()

    with tile.TileContext(nc) as tc:
        singles = tc.alloc_tile_pool(name="singles", bufs=1)
        io = tc.alloc_tile_pool(name="io", bufs=2)
        tqk = tc.alloc_tile_pool(name="tqk", bufs=4)
        ptp = tc.alloc_tile_pool(name="ptp", bufs=2)
        sqp = tc.alloc_tile_pool(name="sqp", bufs=4)
        smalls = tc.alloc_tile_pool(name="smalls", bufs=4)
        vbp = tc.alloc_tile_pool(name="vbp", bufs=2)
        outp = tc.alloc_tile_pool(name="outp", bufs=2)
        stp = tc.alloc_tile_pool(name="stp", bufs=3, space="PSUM")
        otp = tc.alloc_tile_pool(name="otp", bufs=2, space="PSUM")

        ident = singles.tile([128, 128], F32)
        make_identity(nc, ident)
        identr = singles.tile([128, 128], F32R)
        nc.vector.tensor_copy(identr, ident)
        identb = singles.tile([128, 128], BF16)
        nc.vector.tensor_copy(identb, ident)
        # maskneg[n, m] = 0 where m >= n (kept) else MASKVAL; bf16
        trimask = singles.tile([128, 128], F32)
        make_upper_triangular(nc, trimask, val=1.0, diag=True)
        masknegf = singles.tile([128, 128], F32)
        nc.vector.tensor_scalar(
            out=masknegf, in0=trimask, scalar1=-1.0, scalar2=-MASKVAL,
            op0=mybir.AluOpType.add, op1=mybir.AluOpType.mult)
        maskneg = singles.tile([128, 128], BF16)
        nc.vector.tensor_copy(maskneg, masknegf)
        # touch Exp once so the ACT table load happens at t~0, off the
        # first slice's critical path
        actwarm = singles.tile([128, 1], F32)
        nc.scalar.activation(actwarm, ident[:, 0:1], mybir.ActivationFunctionType.Exp)
        # warm the PE p-state during the initial DMA window: ~3.5us of dummy
        # matmuls so the first transposes/QKs run at full clock
        warmsrc = singles.tile([128, 512], BF16)
        nc.vector.memset(warmsrc, 0.0)
        pewarm = stp.tile([128, 512], F32, name="pewarm", tag="st")
        for _ in range(6):
            nc.tensor.matmul(pewarm, identb, warmsrc, start=True, stop=True)

        def emit_in_dma(s):
            """Quarter-tensor tiles for k/q (exact per-tile DMA deps; the
            first transposes can start after ~1/8 of the input traffic),
            half-tensor tiles for v."""
            kn = [io.tile([128, 8, 128], F32R, name=f"kn{s}_{h}", tag="kn")
                  for h in range(2)]
            qn = [io.tile([128, 8, 128], F32R, name=f"qn{s}_{h}", tag="qn")
                  for h in range(2)]
            vn = [io.tile([128, 8, 128], F32, name=f"vn{s}_{h}", tag="vn")
                  for h in range(2)]
            kr = k_dram[s].rearrange("(t p) d -> p t d", p=128)
            qr = q_dram[s].rearrange("(t p) d -> p t d", p=128)
            vr = v_dram[s].rearrange("(t p) d -> p t d", p=128)
            for dst, src in ((kn[0], kr), (qn[0], qr), (vn[0], vr)):
                nc.sync.dma_start(out=dst, in_=src[:, 0:8, :])
            for dst, src in ((kn[1], kr), (qn[1], qr), (vn[1], vr)):
                nc.sync.dma_start(out=dst, in_=src[:, 8:16, :])
            return kn, qn, vn

        class SliceCtx:
            pass

        def start_slice(s, in3, startup=False):
            """Allocate tiles, emit Pool-side prep (squares, vb), and build
            the weave steps (PE transposes + DVE reduces) for slice s."""
            ctx = SliceCtx()
            ctx.s = s
            kn, qn, vn = in3
            ctx.kn, ctx.qn, ctx.vn = kn, qn, vn

            # Pool-side prep, half-granular so each op's DMA dep is exact.
            # Order matters: k h0 squares feed the first exp's bias.
            ctx.sk = [sqp.tile([128, 8, 128], F32, name=f"sq_k{s}_{h}", tag="sq")
                      for h in range(2)]
            ctx.sq = [sqp.tile([128, 8, 128], F32, name=f"sq_q{s}_{h}", tag="sq")
                      for h in range(2)]
            ctx.vb = [vbp.tile([128, 8, 128], BF16, name=f"vb{s}_{h}", tag="vb")
                      for h in range(2)]
            # only the first exp's critical dependency goes on Pool up front;
            # the rest of the Pool prep is woven in need-order
            nc.gpsimd.tensor_mul(ctx.sk[0], kn[0].bitcast(F32), kn[0].bitcast(F32))

            # one tile per half everywhere: deps are tile-granular, so a
            # shared tile would make early readers wait for late half-writes
            ctx.kt = [tqk.tile([128, 1024], BF16, name=f"kt{s}_{h}", tag="kt")
                      for h in range(2)]
            ctx.qt = [tqk.tile([128, 1024], BF16, name=f"qt{s}_{h}", tag="qt")
                      for h in range(2)]
            ctx.ksqb = [smalls.tile([128, 8], F32, name=f"ksqb{s}_{h}", tag="ksqb")
                        for h in range(2)]
            ctx.ksq = [smalls.tile([128, 8], F32, name=f"ksq{s}_{h}", tag="ksq")
                       for h in range(2)]
            ctx.o_out = outp.tile([128, NT, 128], F32, name=f"oout{s}", tag="oout")
            ctx.eq = None

            def tsteps(src, dst, h, nm):
                box = {}

                def step(j):
                    if j == 0:
                        # lazy: allocate at emission so pool rotation follows
                        # emission order (no backward WAR deps in the queues)
                        box["stg"] = stp.tile(
                            [128, 1024], F32R, name=f"tstg{s}_{nm}_{h}", tag="st")
                    stg = box["stg"]
                    nc.tensor.transpose(
                        stg[:, 128 * j : 128 * (j + 1)], src[:, j, :], identr)
                    if j == 7:
                        nc.vector.tensor_copy(dst, stg.bitcast(F32))

                return [lambda j=j: step(j) for j in range(8)]

            def ksq_chain(h):
                """DVE reduce + Pool bias for k half h (squares done on Pool)."""

                def step():
                    nc.vector.tensor_reduce(
                        ctx.ksq[h], ctx.sk[h],
                        axis=mybir.AxisListType.X, op=mybir.AluOpType.add)
                    nc.gpsimd.tensor_scalar_mul(ctx.ksqb[h], ctx.ksq[h], -SM_SCALE)

                return step

            def pool_op(fn, *args):
                return lambda: fn(*args)

            # weave order: everything pass 0 needs first (k/q h0 transposes,
            # vb h0, k_sq h0 bias), then the h1/pass-1 prep in need-order.
            # Startup (slice 0) runs the weave serially before pass 0, so the
            # exp-bias chain goes before the heavy vb/sq Pool ops there.
            if startup:
                # upfront: only what the first QK/exp strictly needs; the
                # rest weaves into this slice's own pass 0 so no heavy Pool
                # op is emitted (and thus counted) ahead of the first exp
                ctx.weave_pre = (
                    tsteps(kn[0], ctx.kt[0], 0, "k") + tsteps(qn[0], ctx.qt[0], 0, "q")
                    + [ksq_chain(0)]
                )
                ctx.weave = (
                    [
                        pool_op(nc.gpsimd.tensor_copy, ctx.vb[0], vn[0]),
                        pool_op(nc.gpsimd.tensor_mul, ctx.sq[0],
                                qn[0].bitcast(F32), qn[0].bitcast(F32)),
                        pool_op(nc.gpsimd.tensor_mul, ctx.sq[1],
                                qn[1].bitcast(F32), qn[1].bitcast(F32)),
                        pool_op(nc.gpsimd.tensor_mul, ctx.sk[1],
                                kn[1].bitcast(F32), kn[1].bitcast(F32)),
                    ]
                    + tsteps(kn[1], ctx.kt[1], 1, "k")
                    + [pool_op(nc.gpsimd.tensor_copy, ctx.vb[1], vn[1])]
                    + tsteps(qn[1], ctx.qt[1], 1, "q")
                    + [ksq_chain(1)]
                )
            else:
                ctx.weave = (
                    tsteps(kn[0], ctx.kt[0], 0, "k") + tsteps(qn[0], ctx.qt[0], 0, "q")
                    + [
                        pool_op(nc.gpsimd.tensor_copy, ctx.vb[0], vn[0]),
                        ksq_chain(0),
                        pool_op(nc.gpsimd.tensor_mul, ctx.sk[1],
                                kn[1].bitcast(F32), kn[1].bitcast(F32)),
                        pool_op(nc.gpsimd.tensor_mul, ctx.sq[0],
                                qn[0].bitcast(F32), qn[0].bitcast(F32)),
                    ]
                    + tsteps(kn[1], ctx.kt[1], 1, "k")
                    + [pool_op(nc.gpsimd.tensor_copy, ctx.vb[1], vn[1])]
                    + tsteps(qn[1], ctx.qt[1], 1, "q")
                    + [
                        ksq_chain(1),
                        pool_op(nc.gpsimd.tensor_mul, ctx.sq[1],
                                qn[1].bitcast(F32), qn[1].bitcast(F32)),
                    ]
                )
            ctx.ptt = {}
            return ctx

        def qk_exp_for(ctx, p):
            """Returns the QK+exp emitter for (slice ctx, pass p). bn == 0
            also allocates the pass's PT tile, so the first call can be used
            as a boundary 'head' from inside the previous pass."""
            s = ctx.s
            mlo = 1024 * p
            bn_hi = 8 if p == 0 else 16

            def qk_exp(bn):
                if bn == 0:
                    ctx.ptt[p] = ptp.tile(
                        [128, bn_hi, 1024], BF16, name=f"pt{s}_{p}", tag="pt")
                m0 = max(128 * bn, mlo)
                w = mlo + 1024 - m0
                c0 = m0 - mlo  # column offset within the pass's q half
                diag = 128 * bn >= mlo
                stt = stp.tile([128, w], F32, name=f"st{s}_{p}_{bn}", tag="st")
                ktb = ctx.kt[bn // 8][:, 128 * (bn % 8) : 128 * (bn % 8 + 1)]
                qth = ctx.qt[p]
                off = 0
                if diag:
                    nc.tensor.matmul(stt[:, 0:128], ktb, qth[:, c0 : c0 + 128],
                                     start=True, stop=False)
                    nc.tensor.matmul(stt[:, 0:128], identb, maskneg,
                                     start=False, stop=True)
                    off = 128
                while off < w:
                    # matmul outputs may not cross a 512-float PSUM bank
                    sw = min(512 - (off % 512), w - off)
                    nc.tensor.matmul(stt[:, off : off + sw], ktb,
                                     qth[:, c0 + off : c0 + off + sw],
                                     start=True, stop=True)
                    off += sw
                nc.scalar.activation(
                    ctx.ptt[p][:, bn, c0 : c0 + w], stt,
                    mybir.ActivationFunctionType.Exp,
                    bias=ctx.ksqb[bn // 8][:, bn % 8 : bn % 8 + 1],
                    scale=2.0 * SM_SCALE)

            return qk_exp

        def emit_pass(ctx, p, weave=None, head=None, skip_n=0, drain_weave=True,
                      pace=2):
            """Emit pass p for slice ctx.
            weave: a shared cursor dict {"items": [...], "i": int} of callables
                   interleaved into the QK stream (filler work), consumed
                   across passes.
            head:  callable invoked just before the last 2 PVs (used to start
                   the NEXT pass/slice's first QK/exp blocks early).
            skip_n: first n QK blocks were already emitted via a boundary head.
            drain_weave: emit all leftover weave items in this pass's tail."""
            s = ctx.s
            mlo = 1024 * p
            bn_hi = 8 if p == 0 else 16
            ptt = ctx.ptt
            oc = {}
            weave = weave if weave is not None else {"items": [], "i": 0}
            qk_exp = qk_exp_for(ctx, p)

            def weave_step():
                if weave["i"] < len(weave["items"]):
                    weave["items"][weave["i"]]()
                    weave["i"] += 1
                    return True
                return False

            def pv(bm):
                cc = (bm - 8 * p) // 4
                j = bm % 4
                if j == 0:
                    oc[cc] = otp.tile([128, 512], F32, name=f"oc{s}_{p}_{cc}", tag="oc")
                col = 128 * bm - mlo
                for bn in range(bm + 1):
                    nc.tensor.matmul(
                        oc[cc][:, 128 * j : 128 * (j + 1)],
                        ptt[p][:, bn, col : col + 128],
                        ctx.vb[bn // 8][:, bn % 8, :],
                        start=(bn == 0), stop=(bn == bm))
                if j == 3:
                    tb = (2 * p + cc) * 4
                    nc.vector.tensor_tensor(
                        out=ctx.o_out[:, tb : tb + 4, :],
                        in0=oc[cc].rearrange("p (t d) -> p t d", t=4),
                        in1=ctx.eq[:, tb : tb + 4].unsqueeze(2).broadcast_to([128, 4, 128]),
                        op=mybir.AluOpType.mult)
                    # stream the finished 512-row chunk out immediately: the
                    # last slice's tail is one chunk, not the whole tensor
                    nc.sync.dma_start(
                        out=o_dram[s].rearrange("(t p) d -> p t d", p=128)[:, tb : tb + 4, :],
                        in_=ctx.o_out[:, tb : tb + 4, :])

            def emit_eq():
                # eq = exp(-s*q_sq): emitted mid-p0 (first consumer is PV(3)'s
                # drain) so its DVE reduce deps can't head-of-line block the
                # exp stream.
                qsq = smalls.tile([128, NT], F32, name=f"qsq{s}", tag="qsq")
                for h in range(2):
                    nc.vector.tensor_reduce(
                        qsq[:, 8 * h : 8 * (h + 1)], ctx.sq[h],
                        axis=mybir.AxisListType.X, op=mybir.AluOpType.add)
                ctx.eq = smalls.tile([128, NT], F32, name=f"eq{s}", tag="eq")
                nc.scalar.activation(
                    ctx.eq, qsq, mybir.ActivationFunctionType.Exp, scale=-SM_SCALE)

            pv_list = list(range(8 * p, 8 * p + 8))
            for bn in range(skip_n, 1):
                qk_exp(bn)
            pvi = 0
            for bn in range(max(1, skip_n), bn_hi):
                for _ in range(pace):
                    weave_step()
                qk_exp(bn)
                if p == 0 and bn == 4:
                    emit_eq()
                while pvi < len(pv_list) and pv_list[pvi] <= bn - 2:
                    pv(pv_list[pvi])
                    pvi += 1
            if drain_weave:
                while weave_step():
                    pass
            # start the next pass/slice's QK+exp before our last PVs so the
            # ACT queue never drains at the boundary
            if head is not None:
                head()
            while pvi < len(pv_list):
                pv(pv_list[pvi])
                pvi += 1

        # --- main pipeline over slices -------------------------------------
        in_tiles = {0: emit_in_dma(0)}
        ctx = start_slice(0, in_tiles.pop(0), startup=True)
        for st in ctx.weave_pre:  # slice 0: minimal upfront prep
            st()
        own_cursor = {"items": ctx.weave, "i": 0}
        for s in range(SLICES):
            if s + 1 < SLICES:
                in_tiles[s + 1] = emit_in_dma(s + 1)

            def two_blocks(c, p):
                qk = qk_exp_for(c, p)
                qk(0)
                qk(1)

            # pass 0, with pass 1's first 2 QK/exp blocks emitted before the
            # last PVs; slice 0 weaves its own remaining prep here
            emit_pass(ctx, 0, skip_n=(2 if s > 0 else 0),
                      weave=(own_cursor if s == 0 else None), pace=4,
                      head=lambda ctx=ctx: two_blocks(ctx, 1))
            if s + 1 < SLICES:
                nxt = start_slice(s + 1, in_tiles.pop(s + 1))
                # pass 1, weaving in the next slice's prep; at the tail,
                # start the next slice's first 2 pass-0 QK/exp blocks
                emit_pass(ctx, 1, weave={"items": nxt.weave, "i": 0}, pace=4,
                          skip_n=2, head=lambda nxt=nxt: two_blocks(nxt, 0))
            else:
                nxt = None
                emit_pass(ctx, 1, skip_n=2)
            ctx = nxt

        for pool in (otp, stp, outp, vbp, smalls, sqp, ptp, tqk, io, singles):
            pool.release()

    nc.compile()
    return nc


def _get_nc():
    global _nc_cache
    if _nc_cache is None:
        _nc_cache = _build_nc()
    return _nc_cache


def run(q, k, v, trace=False):
    q = np.ascontiguousarray(np.asarray(q, dtype=np.float32))
    k = np.ascontiguousarray(np.asarray(k, dtype=np.float32))
    v = np.ascontiguousarray(np.asarray(v, dtype=np.float32))
    qf = q.reshape(B * H, N, D)
    kf = k.reshape(B * H, N, D)
    vf = v.reshape(B * H, N, D)
    nc = _get_nc()
    in_maps = [
        {
            "q": np.ascontiguousarray(qf[SLICES * i : SLICES * (i + 1)]),
            "k": np.ascontiguousarray(kf[SLICES * i : SLICES * (i + 1)]),
            "v": np.ascontiguousarray(vf[SLICES * i : SLICES * (i + 1)]),
        }
        for i in range(NCORES)
    ]
    res = run_bass_kernel_spmd(nc, in_maps, core_ids=list(range(NCORES)), trace=trace)
    out = np.concatenate([res.results[i]["o"] for i in range(NCORES)], axis=0)
    return out.reshape(B, H, N, D).astype(np.float32), res


def kernel(q, k, v):
    return run(q, k, v)[0]
